# revision 21
# baseline (speedup 1.0000x reference)
"""Trainium2 Bass kernel: causal self-attention with RoPE + GQA (+ att matrix output).

Shapes (hardcoded): x (2, 2048, 1024), 16 q-heads / 4 kv-heads, head_size 64.
Sharding: one (batch, kv-head-group) pair per NeuronCore -> 8 cores.
Each core computes its 4 q-heads' attention rows (q,k layout), the attention
output y for those heads, and a partial projection; host sums the 4 partial
projections per batch and adds bias.

Per-core pipeline:
  phase A: qkv projection (fp32r matmuls, x^T stationary), RoPE on DVE in
           natural layout, PE-transposes to build qT/kT (head-size on
           partitions), v kept natural in fp16.
  chunks:  S = q.k^T (fp32r, 2-head row-packed), causal mask via
           tensor_mask_reduce, single exp pass on ScalarE with fused row-sum
           (accum_out) -> fp16 unnormalized P; normalize with per-partition
           1/D (tensor_scalar 4x); DMA att out with fp16->fp32 cast (SWDGE);
           PE-transpose normalized att tiles -> [k,q] strips; y^T = v^T @ attT
           (col-packed pairs); projection with fp16 weights.
"""

import numpy as np
from contextlib import ExitStack

import concourse.bass as bass
from concourse import bacc
import concourse.mybir as mybir
import concourse.tile as tile
from concourse.bass_utils import run_bass_kernel_spmd

F32 = mybir.dt.float32
F16 = mybir.dt.float16
F32R = mybir.dt.float32r

B, T, C = 2, 2048, 1024
NH, NKV, HS = 16, 4, 64
NREP = NH // NKV          # 4 q-heads per core
NCORES = 8
TT = T // 128             # 16 T-tiles
NCH = T // 512            # 4 q-chunks
ROPE_BASE = 10000.0
import os
_SKIP_ATT_DMA = bool(int(os.environ.get("SKIP_ATT_DMA", "0")))
_STAGE = os.environ.get("KSTAGE", "full")  # a | s | full
_SKIP_MASK = bool(int(os.environ.get("SKIP_MASK", "0")))
_SKIP_ACCUM = bool(int(os.environ.get("SKIP_ACCUM", "0")))


def _emit(tc, io):
    nc = tc.nc
    ctx = tc._emit_ctx

    consts = ctx.enter_context(tc.tile_pool(name="consts", bufs=1))
    ident16 = consts.tile([128, 128], F16)
    nc.sync.dma_start(ident16[:], io["ident16"])
    ident32 = consts.tile([128, 128], F32)
    nc.sync.dma_start(ident32[:], io["ident32"])
    trimask = consts.tile([128, 128], F32)
    nc.sync.dma_start(trimask[:], io["trimask"])
    wproj = consts.tile([128, 2, 1024], F16)
    nc.sync.dma_start(wproj[:], io["wprojT"].rearrange("(n p) c -> p n c", p=128))

    qTp = [consts.tile([128, T], F32R, name=f"qTp{i}", tag=f"qTp{i}") for i in range(2)]
    kT2 = consts.tile([128, T], F32R)                      # k dims duplicated both halves
    vN = consts.tile([128, TT, HS], F16)                  # v natural, fp16
    dacc = consts.tile([128, NREP, TT, NCH], F32)         # exp row-sum partials
    r_all = consts.tile([128, NREP, TT], F32)             # 1/D per (head, qtile)
    dred = consts.tile([128, NREP, TT], F32)
    nc.vector.memset(dacc[:], 0.0)

    # ---------------- phase A: qkv projection + rope + transposes -------------
    with tc.tile_pool(name="pa_sb", bufs=1) as pa, \
         tc.tile_pool(name="pa_rot", bufs=3) as rotp, \
         tc.tile_pool(name="pa_ps", bufs=2, space="PSUM") as paps, \
         tc.tile_pool(name="pa_tp", bufs=2, space="PSUM") as patp:
        cs = pa.tile([128, TT, 160], F32)
        nc.sync.dma_start(cs[:], io["cos5"].rearrange("(n p) c -> p n c", p=128))
        sn = pa.tile([128, TT, 160], F32)
        nc.sync.dma_start(sn[:], io["sin5"].rearrange("(n p) c -> p n c", p=128))
        wqkv = pa.tile([128, 8, 384], F32R)
        nc.sync.dma_start(wqkv[:], io["wqkvT"].rearrange("(n p) d -> p n d", p=128))
        xbig = pa.tile([128, 8, T], F32R)
        nc.sync.dma_start(xbig[:], io["xT"].rearrange("(n p) t -> p n t", p=128))

        for tt in range(TT):
            ps = paps.tile([128, 384], F32)
            for kt in range(8):
                nc.tensor.matmul(
                    ps[:],
                    lhsT=xbig[:, kt, tt * 128:(tt + 1) * 128],
                    rhs=wqkv[:, kt, :],
                    start=(kt == 0), stop=(kt == 7),
                )
            # RoPE on q (cols 0:256) and k (cols 256:320); v (cols 320:384) copied.
            rot = rotp.tile([128, 384], F32, tag="rot")
            ev_in = ps[:, 0:320].rearrange("p (b c two) -> p b c two", b=5, two=2)
            csr = cs[:, tt, :].rearrange("p (b c) -> p b c", b=5)
            snr = sn[:, tt, :].rearrange("p (b c) -> p b c", b=5)
            t1 = rotp.tile([128, 5, 32], F32, tag="t1")
            t2 = rotp.tile([128, 5, 32], F32, tag="t2")
            rot_v = rot[:, 0:320].rearrange("p (b c two) -> p b c two", b=5, two=2)
            # even outputs: e*cos - o*sin
            nc.vector.tensor_tensor(t1[:], ev_in[:, :, :, 0], csr, op=mybir.AluOpType.mult)
            nc.vector.tensor_tensor(t2[:], ev_in[:, :, :, 1], snr, op=mybir.AluOpType.mult)
            nc.vector.tensor_tensor(rot_v[:, :, :, 0], t1[:], t2[:], op=mybir.AluOpType.subtract)
            # odd outputs: o*cos + e*sin
            nc.vector.tensor_tensor(t1[:], ev_in[:, :, :, 1], csr, op=mybir.AluOpType.mult)
            nc.vector.tensor_tensor(t2[:], ev_in[:, :, :, 0], snr, op=mybir.AluOpType.mult)
            nc.vector.tensor_tensor(rot_v[:, :, :, 1], t1[:], t2[:], op=mybir.AluOpType.add)
            # v -> fp16 natural
            nc.scalar.activation(vN[:, tt, :], ps[:, 320:384],
                                 mybir.ActivationFunctionType.Copy)
            # transposes -> qT pairs and duplicated kT
            tq0 = patp.tile([128, 128], F32, tag="tq")
            nc.tensor.transpose(tq0[:], rot[:, 0:128], ident32[:])
            nc.vector.tensor_copy(qTp[0][:, tt * 128:(tt + 1) * 128], tq0[:])
            tq1 = patp.tile([128, 128], F32, tag="tq")
            nc.tensor.transpose(tq1[:], rot[:, 128:256], ident32[:])
            nc.vector.tensor_copy(qTp[1][:, tt * 128:(tt + 1) * 128], tq1[:])
            tka = patp.tile([128, 128], F32, tag="tq")
            nc.tensor.transpose(tka[0:64, :], rot[:, 256:320], ident32[:])
            nc.vector.tensor_copy(kT2[0:64, tt * 128:(tt + 1) * 128], tka[0:64, :])
        # duplicate k dims into partitions 64..127 (partition remap via DMA)
        nc.sync.dma_start(kT2[64:128, :], kT2[0:64, :])

    # ---------------- attention chunks ---------------------------------------
    if _STAGE == "a":
        return
    att_v = io["att"].rearrange("h q k -> q h k")
    with tc.tile_pool(name="attu", bufs=6) as attup, \
         tc.tile_pool(name="strips", bufs=10) as stripp, \
         tc.tile_pool(name="yts", bufs=4) as ytsp, \
         tc.tile_pool(name="ystage", bufs=2) as ystp, \
         tc.tile_pool(name="sps", bufs=4, space="PSUM") as sps, \
         tc.tile_pool(name="tps", bufs=2, space="PSUM") as tps, \
         tc.tile_pool(name="accps", bufs=2, space="PSUM") as accps:
        for j in range(NCH):
            attbig = []
            for s in range(4):
                t = 4 * j + s
                kr = 128 * (t + 1)
                ab = attup.tile([128, NREP, T], F16, tag="attbig")
                attbig.append(ab)
                nfull = t // 4
                for p in range(2):
                    for half in range(2):
                        h = 2 * p + half
                        for c in range(nfull + 1):
                            n = 512 if c < nfull else kr - 512 * nfull
                            sp = sps.tile([128, 512], F32, tag="s")
                            nc.tensor.matmul(
                                sp[:, 0:n],
                                lhsT=qTp[p][64 * half:64 * half + 64,
                                            t * 128:(t + 1) * 128],
                                rhs=kT2[64 * half:64 * half + 64,
                                        512 * c:512 * c + n],
                                start=True, stop=True,
                            )
                            if c == nfull and not _SKIP_MASK:
                                # causal mask on the diagonal 128-col subblock
                                dg = sp[:, n - 128:n]
                                nc.vector.tensor_tensor(
                                    dg, dg, trimask[:],
                                    op=mybir.AluOpType.add,
                                )
                            nc.scalar.activation(
                                ab[:, h, 512 * c:512 * c + n], sp[:, 0:n],
                                mybir.ActivationFunctionType.Exp,
                                scale=0.125,
                                accum_out=(None if _SKIP_ACCUM
                                           else dacc[:, h, t, c:c + 1]),
                            )
            # softmax denominators for this chunk
            for h in range(NREP):
                nc.vector.tensor_reduce(
                    dred[:, h, 4 * j:4 * j + 4], dacc[:, h, 4 * j:4 * j + 4, :],
                    axis=mybir.AxisListType.X, op=mybir.AluOpType.add,
                )
                nc.vector.reciprocal(r_all[:, h, 4 * j:4 * j + 4],
                                     dred[:, h, 4 * j:4 * j + 4])
            # normalize + write out att
            for s in range(4):
                t = 4 * j + s
                kr = 128 * (t + 1)
                for h in range(NREP):
                    nc.vector.tensor_scalar(
                        attbig[s][:, h, 0:kr], attbig[s][:, h, 0:kr],
                        r_all[:, h, t:t + 1], None, op0=mybir.AluOpType.mult,
                    )
                if not _SKIP_ATT_DMA:
                    nc.gpsimd.dma_start(
                        out=att_v[t * 128:(t + 1) * 128, :, 0:kr],
                        in_=attbig[s][:, :, 0:kr],
                    )
            if _STAGE == "s":
                continue
            # per head: transpose att tiles to [k,q] strips, y^T = v^T @ att^T.
            # yT pair tiles are assembled via DMA partition remap for odd heads
            # (engines cannot move data across partitions).
            ytile = [ytsp.tile([128, 512], F16, name=f"yt{pp}", tag=f"yt{pp}")
                     for pp in range(2)]
            for h in range(NREP):
                yps = accps.tile([64, 512], F32, tag="ypsum")
                for i in range(4 * j + 4):
                    smin = max(0, i - 4 * j)
                    off = 128 * smin
                    tp = tps.tile([128, 512], F16, tag="tp")
                    for s in range(smin, 4):
                        nc.tensor.matmul(
                            tp[:, s * 128:(s + 1) * 128],
                            lhsT=attbig[s][:, h, i * 128:(i + 1) * 128],
                            rhs=ident16[:],
                            is_transpose=True,
                            start=(s == smin), stop=(s == 3),
                            skip_group_check=True,
                        )
                    st = stripp.tile([128, 512], F16, tag="strip")
                    nc.vector.tensor_copy(st[:, off:512], tp[:, off:512])
                    nc.tensor.matmul(
                        yps[:, off:512],
                        lhsT=vN[:, i, :],
                        rhs=st[:, off:512],
                        start=(i == 0), stop=(i == 4 * j + 3),
                        skip_group_check=True,
                    )
                p, half = h // 2, h % 2
                if half == 0:
                    nc.vector.tensor_copy(ytile[p][0:64, :], yps[:])
                else:
                    yh16 = ytsp.tile([64, 512], F16, tag="ytmp")
                    nc.vector.tensor_copy(yh16[:], yps[:])
                    nc.sync.dma_start(ytile[p][64:128, :], yh16[:])
            # projection for this chunk's 4 q-tiles
            for s in range(4):
                t = 4 * j + s
                yst = ystp.tile([128, 1024], F32, tag="yst")
                for cc in range(2):
                    pp = sps.tile([128, 512], F32, tag="s")
                    for p in range(2):
                        nc.tensor.matmul(
                            pp[:],
                            lhsT=ytile[p][:, s * 128:(s + 1) * 128],
                            rhs=wproj[:, p, 512 * cc:512 * cc + 512],
                            start=(p == 0), stop=(p == 1),
                        )
                    nc.scalar.activation(yst[:, 512 * cc:512 * cc + 512], pp[:],
                                         mybir.ActivationFunctionType.Copy)
                nc.sync.dma_start(io["ypart"][t * 128:(t + 1) * 128, :], yst[:])


def _build():
    nc = bacc.Bacc("TRN2", target_bir_lowering=False, debug=False,
                   num_devices=NCORES)
    io = {}
    io["xT"] = nc.dram_tensor("xT", [C, T], F32R, kind="ExternalInput").ap()
    io["wqkvT"] = nc.dram_tensor("wqkvT", [C, 384], F32R, kind="ExternalInput").ap()
    io["wprojT"] = nc.dram_tensor("wprojT", [256, C], F16, kind="ExternalInput").ap()
    io["cos5"] = nc.dram_tensor("cos5", [T, 160], F32, kind="ExternalInput").ap()
    io["sin5"] = nc.dram_tensor("sin5", [T, 160], F32, kind="ExternalInput").ap()
    io["ident16"] = nc.dram_tensor("ident16", [128, 128], F16, kind="ExternalInput").ap()
    io["ident32"] = nc.dram_tensor("ident32", [128, 128], F32, kind="ExternalInput").ap()
    io["trimask"] = nc.dram_tensor("trimask", [128, 128], F32, kind="ExternalInput").ap()
    io["att"] = nc.dram_tensor("att", [NREP, T, T], F32, kind="ExternalOutput").ap()
    io["ypart"] = nc.dram_tensor("ypart", [T, C], F32, kind="ExternalOutput").ap()
    with tile.TileContext(nc) as tc:
        with ExitStack() as ctx:
            tc._emit_ctx = ctx
            _emit(tc, io)
    nc.compile()
    return nc


def _host_inputs(x, Wq, Wk, Wv, Wproj):
    inv = 1.0 / (ROPE_BASE ** (np.arange(0, HS, 2, dtype=np.float32) / HS))
    tpos = np.arange(T, dtype=np.float32)
    fr = np.outer(tpos, inv)
    cos = np.cos(fr).astype(np.float32)
    sin = np.sin(fr).astype(np.float32)
    cos5 = np.ascontiguousarray(np.tile(cos, (1, 5)))
    sin5 = np.ascontiguousarray(np.tile(sin, (1, 5)))
    ident16 = np.eye(128, dtype=np.float16)
    ident32 = np.eye(128, dtype=np.float32)
    trimask = np.where(np.arange(128)[None, :] <= np.arange(128)[:, None],
                       np.float32(0.0), np.float32(-1e30)).astype(np.float32)
    in_maps = []
    for core in range(NCORES):
        b, g = core // NKV, core % NKV
        wqkvT = np.concatenate(
            [Wq[256 * g:256 * (g + 1)], Wk[64 * g:64 * (g + 1)],
             Wv[64 * g:64 * (g + 1)]], axis=0).T
        in_maps.append({
            "xT": np.ascontiguousarray(x[b].T),
            "wqkvT": np.ascontiguousarray(wqkvT),
            "wprojT": np.ascontiguousarray(
                Wproj[:, 256 * g:256 * (g + 1)].T.astype(np.float16)),
            "cos5": cos5,
            "sin5": sin5,
            "ident16": ident16,
            "ident32": ident32,
            "trimask": trimask,
        })
    return in_maps


_NC = None


def _gather(res, bproj):
    att = np.empty((B, NH, T, T), dtype=np.float32)
    y = np.zeros((B, T, C), dtype=np.float32)
    for core in range(NCORES):
        b, g = core // NKV, core % NKV
        att[b, NREP * g:NREP * (g + 1)] = res[core]["att"]
        y[b] += res[core]["ypart"]
    y += np.asarray(bproj, dtype=np.float32)
    return y, att


def kernel(x, Wq, Wk, Wv, Wproj, bproj):
    global _NC
    x = np.asarray(x, dtype=np.float32)
    Wq = np.asarray(Wq, dtype=np.float32)
    Wk = np.asarray(Wk, dtype=np.float32)
    Wv = np.asarray(Wv, dtype=np.float32)
    Wproj = np.asarray(Wproj, dtype=np.float32)
    if _NC is None:
        _NC = _build()
    in_maps = _host_inputs(x, Wq, Wk, Wv, Wproj)
    res = run_bass_kernel_spmd(_NC, in_maps, list(range(NCORES))).results
    return _gather(res, bproj)
